# revision 10
# baseline (speedup 1.0000x reference)
"""Batch-hard triplet loss on 8 TRN2 cores — symmetric Gram scheme.

Each unordered 512x512 block-pair of the Gram matrix is computed ONCE
(circulant assignment: core c computes blocks (c, c+k mod 8), k=0..4; the
antipodal k=4 pair is computed twice for uniformity).  Core c:

  mm(b):  t = x[rows_c] . x[rows_{c+b}]^T  (fp8 DoubleRow, K=2048)
  rm(b):  row-mine: w = t - cmb_b (fused custom-DVE sub+row-min -> pos
          partial) then row-max via tensor_scalar accumulate (neg partial);
          cmb_b = sq_col/2 + 4096*same carries the identity mask, so the
          same-id set always wins the min and never the max.
  T(b), cm(b) for b=1..3: PE-transpose w (fp16, via identity matmul) and
          mine the transposed tiles with in1 = sq_own/2: values become
          -d^2/2 - 4096*same, so the same row-min/row-max pair yields the
          partner rows' pos/neg partials over this core's columns.

Host combines per-row partials from the owning core (5 blocks, staged-2
on device) and from 3 remote cores' transposed minings, then does
sqrt/relu/mean.  Rows with no same-id entry in a remote block produce a
candidate <= max_d^2 - 8192 < 0 which can never win the host-side max.
"""

import numpy as np
import ml_dtypes

MARGIN = 0.2
C_BIG = 4096.0


def _register_sub_min():
    """Custom DVE op: out = in0 - in1 (fp16), accum_out = min(s0, row-min)."""
    import concourse.dve_ops as dve_ops
    from concourse.dve_spec import Spec, Src0, Src1, C0
    from concourse.dve_uop import DveOpSpec
    from concourse.dve_ops import DveOp, lower, has_src1, minn

    name = "SUB_MIN_ANT_K77"
    if name in dve_ops._SUB_OPCODE_FOR_NAME:
        return next(op for op in dve_ops.OPS if op.name == name)
    spec = Spec(body=Src0 - Src1, accum=minn, accum_init=C0)
    opcode = dve_ops._CUSTOM_DVE_ROW_BASE + len(dve_ops.OPS)
    assert opcode < 0x20
    shas = {}
    for ver in ("v3", "v4"):
        s = DveOpSpec(name=name, opcode=opcode, uops=lower(spec, ver=ver),
                      rd1_en=has_src1(spec))
        shas[ver] = s.sha(ver)
    op = DveOp(name, spec, subdim=False, uops_sha=shas)
    dve_ops.OPS.append(op)
    dve_ops._SUB_OPCODE_FOR_NAME[name] = opcode
    dve_ops.CUSTOM_DVE_SPECS[name] = spec
    return op


class _Cfg:
    def __init__(self, n=4096, d=2048, nids=256, ncores=8):
        self.n, self.d, self.nids, self.ncores = n, d, nids, ncores
        self.m = n // ncores            # 512 rows per core
        self.K = d
        self.KP = d // 256              # 8 DoubleRow pair-chunks
        self.MCH = self.m // 128        # 4 row chunks
        self.NBLK = 5                   # col blocks c..c+4
        self.NT = 3                     # transposed (col-mined) blocks 1..3


_DEFAULT = _Cfg()


def _build_program(cfg: _Cfg):
    from contextlib import ExitStack

    import concourse.bacc as bacc
    import concourse.mybir as mybir
    from concourse import tile, masks

    f32 = mybir.dt.float32
    f16 = mybir.dt.float16
    fp8 = mybir.dt.float8e4
    Alu = mybir.AluOpType
    AxX = mybir.AxisListType.X
    DR = mybir.MatmulPerfMode.DoubleRow
    sub_min = _register_sub_min()

    nc = bacc.Bacc(
        "TRN2", target_bir_lowering=False, debug=False, num_devices=cfg.ncores
    )

    ut0_h = nc.dram_tensor("ut0", [cfg.KP, 128, 2, 512], fp8,
                           kind="ExternalInput")
    utr_h = nc.dram_tensor("utr", [cfg.NBLK - 1, 128, cfg.KP, 2, 512], fp8,
                           kind="ExternalInput")
    cmb_h = nc.dram_tensor("cmb", [cfg.NBLK, 128, cfg.MCH, 512], mybir.dt.float16,
                           kind="ExternalInput")
    hsq_h = nc.dram_tensor("hsq", [128, 512], mybir.dt.float16,
                           kind="ExternalInput")
    # out[:, 0]    = own-row (minw, maxw) per mi  (w-domain)
    # out[:, 1..3] = partner rows of core c+b, (minv, maxv) per q (v-domain)
    out_h = nc.dram_tensor("out", [128, 4, cfg.MCH, 2], f32,
                           kind="ExternalOutput")

    with tile.TileContext(nc) as tc, ExitStack() as ctx:
        u0_pool = ctx.enter_context(tc.tile_pool(name="u0", bufs=cfg.KP))
        ur_pool = ctx.enter_context(tc.tile_pool(name="ur", bufs=cfg.NBLK - 1))
        cmb_pool = ctx.enter_context(tc.tile_pool(name="cmb", bufs=1))
        cst_pool = ctx.enter_context(tc.tile_pool(name="cst", bufs=1))
        w_pool = ctx.enter_context(tc.tile_pool(name="w", bufs=2 * cfg.MCH))
        wd_pool = ctx.enter_context(tc.tile_pool(name="wd", bufs=2))
        ps_pool = ctx.enter_context(
            tc.tile_pool(name="ps", bufs=6, space="PSUM"))
        pt_pool = ctx.enter_context(
            tc.tile_pool(name="pt", bufs=2, space="PSUM"))

        u0_tiles = []
        for kp in range(cfg.KP):
            u_t = u0_pool.tile([128, 2, 512], fp8, tag="u0", name=f"u0_{kp}")
            nc.sync.dma_start(u_t[:], ut0_h.ap()[kp])
            u0_tiles.append(u_t)
        # scalar-queue order: utr block 1 (needed first by the PE), then the
        # whole cmb tile (gates the first row-mine -- whole-tile readiness),
        # then the remaining utr blocks, which the PE needs much later.
        ur_tiles = []
        u_t = ur_pool.tile([128, cfg.KP, 2, 512], fp8, tag="ur", name="ur_1")
        nc.scalar.dma_start(u_t[:], utr_h.ap()[0])
        ur_tiles.append(u_t)
        cmb_sb = cmb_pool.tile([128, cfg.NBLK, cfg.MCH, 512], f16, tag="cmb")
        for b in range(cfg.NBLK):
            nc.scalar.dma_start(cmb_sb[:, b], cmb_h.ap()[b])
        hsq_sb = cst_pool.tile([128, 512], f16, tag="hsq")
        nc.scalar.dma_start(hsq_sb[:], hsq_h.ap())
        for b in range(2, cfg.NBLK):
            u_t = ur_pool.tile([128, cfg.KP, 2, 512], fp8, tag="ur",
                               name=f"ur_{b}")
            nc.scalar.dma_start(u_t[:], utr_h.ap()[b - 1])
            ur_tiles.append(u_t)

        ident = cst_pool.tile([128, 128], f16, tag="ident")
        masks.make_identity(nc, ident[:])

        minw_sb = cst_pool.tile([128, cfg.MCH, 8], f32, tag="minw")
        maxw_sb = cst_pool.tile([128, cfg.MCH, 8], f32, tag="maxw")
        out_sb = cst_pool.tile([128, 4, cfg.MCH, 2], f32, tag="out")

        def lhsT(kp, mi):
            return u0_tiles[kp][:, :, mi * 128:(mi + 1) * 128]

        def rhs(b, kp):
            if b == 0:
                return u0_tiles[kp][:]
            return ur_tiles[b - 1][:, kp]

        w16_of = {}  # (b, mi) -> w16 tile

        def rmine(b, mi, ps, split=False):
            w16 = w_pool.tile([128, 512], f16, tag="w", name=f"w{b}_{mi}")
            w16_of[(b, mi)] = w16
            if split:
                for h in range(2):
                    sl = slice(h * 256, (h + 1) * 256)
                    nc.vector._custom_dve(
                        sub_min, out=w16[:, sl], in0=ps[:, sl],
                        in1=cmb_sb[:, b, mi, sl], s0=0.0,
                        accum_out=minw_sb[:, mi, b + h:b + h + 1])
                    nc.vector.tensor_reduce(
                        maxw_sb[:, mi, b + h:b + h + 1], w16[:, sl],
                        axis=AxX, op=Alu.max)
            else:
                nc.vector._custom_dve(
                    sub_min, out=w16[:], in0=ps[:],
                    in1=cmb_sb[:, b, mi], s0=0.0,
                    accum_out=minw_sb[:, mi, b:b + 1])
                nc.vector.tensor_reduce(
                    maxw_sb[:, mi, b:b + 1], w16[:], axis=AxX, op=Alu.max)

        def mm_rm(b, split_last=False):
            """Matmul block b (4 PSUM chunks) + row-mine each chunk.

            Block 0 runs kp-outer (its per-kp tiles stream in); later
            blocks run mi-outer so each chunk finishes (and mines) early.
            """
            ps = [ps_pool.tile([128, 512], f32, tag="ps",
                               name=f"ps{b}_{mi}") for mi in range(cfg.MCH)]
            for mi in range(cfg.MCH):
                for kp in range(cfg.KP):
                    nc.tensor.matmul(
                        ps[mi][:], lhsT(kp, mi), rhs(b, kp),
                        start=(kp == 0), stop=(kp == cfg.KP - 1),
                        perf_mode=DR,
                    )
                rmine(b, mi, ps[mi],
                      split=(split_last and mi == cfg.MCH - 1))

        def t_cm(b):
            """PE-transpose block b's w16 and col-mine -> partner partials."""
            for q in range(cfg.MCH):
                psT = pt_pool.tile([128, 512], f16, tag="pst",
                                   name=f"pst{b}_{q}")
                for mi in range(cfg.MCH):
                    nc.tensor.transpose(
                        psT[:, mi * 128:(mi + 1) * 128],
                        w16_of[(b, mi)][:, q * 128:(q + 1) * 128],
                        ident[:])
                vt = w_pool.tile([128, 512], f16, tag="vt",
                                 name=f"vt{b}_{q}")
                nc.vector._custom_dve(
                    sub_min, out=vt[:], in0=psT[:], in1=hsq_sb[:],
                    s0=0.0, accum_out=out_sb[:, b, q, 0:1])
                nc.vector.tensor_reduce(
                    out_sb[:, b, q, 1:2], vt[:], axis=AxX, op=Alu.max)

        # PE order: mm0 mm1 mm2 T1 mm3 T2 mm4 T3 (all blocks mi-outer, so
        # each chunk's mining staggers); DVE order: rm0..rm2 cm1 rm3 cm2
        # rm4 st2 cm3.  Own-row partials ship as soon as rm4 finishes; the
        # pipelined T3(q)/cm3(q) pair forms the (short) tail.
        mm_rm(0)
        mm_rm(1)
        mm_rm(2)
        t_cm(1)
        mm_rm(3)
        t_cm(2)
        mm_rm(4, split_last=True)

        for mi in range(cfg.MCH):
            e = 6 if mi == cfg.MCH - 1 else 5
            nc.vector.tensor_reduce(out_sb[:, 0, mi, 0:1],
                                    minw_sb[:, mi, 0:e], axis=AxX, op=Alu.min)
            nc.vector.tensor_reduce(out_sb[:, 0, mi, 1:2],
                                    maxw_sb[:, mi, 0:e], axis=AxX, op=Alu.max)
        nc.scalar.dma_start(out_h.ap()[:, 0:2], out_sb[:, 0:2])

        t_cm(3)
        nc.scalar.dma_start(out_h.ap()[:, 2:4], out_sb[:, 2:4])

    nc.compile()
    return nc


# --------------------------------------------------------------------------
# host-side prep + combine
# --------------------------------------------------------------------------

def _prep_inputs(feature: np.ndarray, identity: np.ndarray, cfg: _Cfg):
    e4 = ml_dtypes.float8_e4m3
    n, d, ncores, m = cfg.n, cfg.d, cfg.ncores, cfg.m

    feature = np.asarray(feature, dtype=np.float32)
    identity = np.asarray(identity).astype(np.int64).ravel()
    assert feature.shape == (n, d) and identity.shape == (n,)

    x8 = feature.astype(e4)
    sq = np.einsum("ij,ij->i", feature, feature, dtype=np.float32)
    hsq = (0.5 * sq).astype(np.float32)

    in_maps = []
    for c in range(ncores):
        rows = slice(c * m, (c + 1) * m)
        u_c = np.roll(x8, -c * m, axis=0)[:cfg.NBLK * m]   # [2560, 2048]
        # block 0 per-kp tiles
        u0 = u_c[0:m]                                      # [512, 2048]
        ut0 = np.ascontiguousarray(
            u0.T.reshape(cfg.KP, 2, 128, m).transpose(0, 2, 1, 3))
        # blocks 1..4, one 8KB-line transfer each
        utr = np.stack([
            np.ascontiguousarray(
                u_c[b * m:(b + 1) * m].T
                .reshape(cfg.KP, 2, 128, m).transpose(2, 0, 1, 3))
            for b in range(1, cfg.NBLK)])                  # [4, 128, KP, 2, 512]

        ids_r = np.roll(identity, -c * m)[:cfg.NBLK * m]
        same = identity[rows][:, None] == ids_r[None, :]   # [512, 2560]
        hs_r = np.roll(hsq, -c * m)[:cfg.NBLK * m]
        cmb = (hs_r[None, :] + np.float32(C_BIG) * same).astype(np.float16)
        cmb = np.ascontiguousarray(
            cmb.reshape(cfg.MCH, 128, cfg.NBLK, m).transpose(2, 1, 0, 3))

        hsq_own = np.ascontiguousarray(np.broadcast_to(
            hsq[rows].astype(np.float16)[None, :], (128, m)))

        in_maps.append({"ut0": ut0, "utr": np.ascontiguousarray(utr),
                        "cmb": cmb, "hsq": hsq_own})
    return in_maps


_PROGRAM_CACHE: dict = {}
_LAST_RESULTS = None


def _get_program(cfg: _Cfg):
    key = (cfg.n, cfg.d, cfg.nids, cfg.ncores)
    if key not in _PROGRAM_CACHE:
        _PROGRAM_CACHE[key] = _build_program(cfg)
    return _PROGRAM_CACHE[key]


def _run_once(feature, identity, _trace):
    global _LAST_RESULTS
    from concourse.bass_utils import run_bass_kernel_spmd

    cfg = _DEFAULT
    nc = _get_program(cfg)
    feature = np.asarray(feature, dtype=np.float32)
    identity = np.asarray(identity).astype(np.int64).ravel()
    in_maps = _prep_inputs(feature, identity, cfg)
    res = run_bass_kernel_spmd(
        nc, in_maps, list(range(cfg.ncores)), trace=_trace)
    _LAST_RESULTS = res

    n, m = cfg.n, cfg.m
    sq = np.einsum("ij,ij->i", feature, feature,
                   dtype=np.float32).astype(np.float64)
    pos2 = np.full(n, -np.inf)
    neg2 = np.full(n, np.inf)
    for c in range(cfg.ncores):
        o = np.asarray(res.results[c]["out"], dtype=np.float64)  # [128,4,MCH,2]
        # own rows (w-domain): row = c*m + mi*128 + p
        minw = o[:, 0, :, 0].T.ravel()
        maxw = o[:, 0, :, 1].T.ravel()
        r = np.arange(c * m, (c + 1) * m)
        pos2[r] = np.maximum(pos2[r], sq[r] - 2 * minw - 2 * C_BIG)
        neg2[r] = np.minimum(neg2[r], sq[r] - 2 * maxw)
        # partner rows (v-domain): rows of core (c+b) mod 8
        for b in range(1, 4):
            minv = o[:, b, :, 0].T.ravel()
            maxv = o[:, b, :, 1].T.ravel()
            rp = (np.arange(m) + ((c + b) % cfg.ncores) * m)
            pos2[rp] = np.maximum(pos2[rp], -2 * minv - 2 * C_BIG)
            neg2[rp] = np.minimum(neg2[rp], -2 * maxv)
    pos_d = np.sqrt(np.maximum(pos2, 0.0))
    neg_d = np.sqrt(np.maximum(neg2, 0.0))
    return float(np.maximum(MARGIN + pos_d - neg_d, 0.0).sum())


def _subprocess_worker(path, feature, identity, q):
    import importlib.util
    spec = importlib.util.spec_from_file_location("_kernel_sub", path)
    mod = importlib.util.module_from_spec(spec)
    spec.loader.exec_module(mod)
    q.put(mod._run_once(feature, identity, False))


def kernel(feature, identity, epoch=None, _trace=False):
    cfg = _DEFAULT
    last_err = None
    for attempt in range(2):
        try:
            total = _run_once(feature, identity, _trace)
            if not np.isfinite(total):
                raise FloatingPointError(f"non-finite loss {total}")
            return np.float32(total / cfg.n)
        except Exception as e:
            last_err = e
            import time
            time.sleep(3.0 * (attempt + 1))
    try:
        import multiprocessing as mp
        ctx = mp.get_context("spawn")
        q = ctx.Queue()
        p = ctx.Process(target=_subprocess_worker,
                        args=(__file__, np.asarray(feature),
                              np.asarray(identity), q))
        p.start()
        total = q.get(timeout=900)
        p.join(timeout=30)
        return np.float32(total / cfg.n)
    except Exception:
        raise last_err
